# revision 1
# baseline (speedup 1.0000x reference)
"""CNF forward (vector field + exact Jacobian trace) on 8 TRN2 cores.

Math: reference computes, per sample x (row of state[:, 1:]):
    f(x)  = W3^T tanh(W2^T tanh(W1^T [x; t] + b1) + b2) + b3      (dx)
    trJ   = trace(df/dx)                                          (aug = -trJ)

Instead of D=64 JVPs per sample, use the closed form of the trace:
    h1 = tanh([x;t] @ W1 + b1),  h2 = tanh(h1 @ W2 + b2)
    s1 = 1 - h1^2,               s2 = 1 - h2^2
    trJ = s1^T F s2   with  F[h',h] = W2[h',h] * (W3 @ W1[:D])[h, h']
F depends only on the weights and is computed on-device per core
(one K=64 matmul per 128-row tile + an elementwise multiply).

Sharding: data-parallel, 128 samples per core, weights replicated.

Host-side work is layout-only (sharding, zero-FLOP transposes, packing
t/b1 into one bias block); all FLOPs run on device. Layer 1 runs
feature-major (h1T tiles) so W1 itself is the matmul lhsT; layers 2/3
and the trace matmul run batch-major with feature-major activations as
lhsT, giving N=512 fp32 matmuls and no on-device weight transposes.
"""

import numpy as np

import concourse.bacc as bacc
import concourse.bass as bass
import concourse.tile as tile
from concourse import mybir
from concourse.bass_utils import run_bass_kernel_spmd
from concourse.masks import make_identity
from concourse.tile_rust import add_dep_helper

B, D, H = 1024, 64, 512
NCORES = 8
BC = B // NCORES  # 128 samples per core
KT = H // 128     # 4 feature tiles of 128
F32 = mybir.dt.float32
AF = mybir.ActivationFunctionType
ALU = mybir.AluOpType
ts = bass.ts

_NC = {}

USE_DIST_F = False  # AllGather costs ~70us in this env - keep per-core

# (engine, tensor) load order; engines: sync=SP HWDGE, scalar=Act HWDGE
DMA_PLAN = [
    ("scalar", "stT"), ("scalar", "w1x"), ("scalar", "cblk"),
    ("sync", "w2_0"), ("scalar", "w2_1"), ("sync", "w2_2"),
    ("scalar", "w2_3"), ("sync", "w3T"),
    ("sync", "w3_0"), ("sync", "w3_1"), ("sync", "w3_2"),
    ("sync", "w3_3"),
]


def _build(with_bias23: bool):
    """with_bias23: include rank-1 bias adds for b2/b3 (batch-major layers
    can't take a per-free-dim bias via ACT). setup_inputs() has zero
    biases so the fast path skips them; nonzero b2/b3 still works."""
    nc = bacc.Bacc()

    stT = nc.declare_dram_parameter("stT", [D, BC], F32, isOutput=False)
    W1x = nc.declare_dram_parameter("W1x", [D, H], F32, isOutput=False)
    W2 = nc.declare_dram_parameter("W2", [H, H], F32, isOutput=False)
    W3 = nc.declare_dram_parameter("W3", [H, D], F32, isOutput=False)
    W3T = nc.declare_dram_parameter("W3T", [D, H], F32, isOutput=False)
    # packed constants: cols 0-3 = b1 + t*W1[D] per feature tile
    cblk = nc.declare_dram_parameter("cblk", [128, KT], F32, isOutput=False)
    if USE_DIST_F:
        # this core's 64-column slice of W1[:D] (columns c*64:(c+1)*64)
        w1me = nc.declare_dram_parameter("w1me", [D, D], F32, isOutput=False)
    if with_bias23:
        b2r = nc.declare_dram_parameter("b2r", [1, H], F32, isOutput=False)
        b3r = nc.declare_dram_parameter("b3r", [1, D], F32, isOutput=False)
    out = nc.declare_dram_parameter("out", [BC, D + 1], F32, isOutput=True)

    with tile.TileContext(nc) as tc:
        with (
            tc.tile_pool(name="const", bufs=1) as cp,
            tc.tile_pool(name="act", bufs=1) as ap,
            tc.tile_pool(name="ps", bufs=1, space="PSUM") as ps,
            tc.tile_pool(name="dram", bufs=1, space="DRAM") as dp,
        ):
            # ------------- loads (plan set by DMA_PLAN) -------------
            stT_sb = ap.tile([D, BC], F32, tag="stT")
            w1x = cp.tile([D, H], F32, tag="w1x")
            cblk_sb = cp.tile([128, KT], F32, tag="cblk")
            w2_sb = [cp.tile([128, H], F32, tag=f"w2_{k}", name=f"w2_{k}")
                     for k in range(KT)]
            w3T_sb = cp.tile([D, H], F32, tag="w3T")
            w3_sb = [cp.tile([128, D], F32, tag=f"w3_{k}", name=f"w3_{k}")
                     for k in range(KT)]
            srcs = {"stT": (stT_sb, stT), "w1x": (w1x, W1x),
                    "cblk": (cblk_sb, cblk), "w3T": (w3T_sb, W3T)}
            for k in range(KT):
                srcs[f"w2_{k}"] = (w2_sb[k], W2[ts(k, 128), :])
                srcs[f"w3_{k}"] = (w3_sb[k], W3[ts(k, 128), :])
            for eng, nm in DMA_PLAN:
                dst, src = srcs[nm]
                src = src if isinstance(src, bass.AP) else src[:, :]
                getattr(nc, eng).dma_start(out=dst, in_=src)
            if with_bias23:
                b2r_sb = cp.tile([1, H], F32, tag="b2r")
                nc.sync.dma_start(out=b2r_sb, in_=b2r[:, :])
                b3r_sb = cp.tile([1, D], F32, tag="b3r")
                nc.sync.dma_start(out=b3r_sb, in_=b3r[:, :])
                onesr = cp.tile([1, BC], F32, tag="onesr")
                nc.vector.memset(onesr, 1.0)
            ident = cp.tile([128, 128], F32, tag="ident")
            make_identity(nc, ident)

            # ------------- layer 1 (feature-major): h1T, s1T -------------
            h1, s1, z1_mm = [], [], []
            for j in range(KT):
                z1_ps = ps.tile([128, BC], F32, tag="z1", bufs=2)
                z1_mm.append(
                    nc.tensor.matmul(z1_ps, w1x[:, ts(j, 128)],
                                     stT_sb, start=True, stop=True))
                h = ap.tile([128, BC], F32, tag=f"h1_{j}")
                nc.scalar.activation(h, z1_ps, AF.Tanh,
                                     bias=cblk_sb[:, j:j + 1])
                s = ap.tile([128, BC], F32, tag=f"s1_{j}")
                nc.gpsimd.tensor_mul(s, h, h)
                nc.gpsimd.tensor_scalar(s, s, -1.0, 1.0, ALU.mult, ALU.add)
                h1.append(h)
                s1.append(s)

            # ------------- layer 2 (batch-major): h2, s2 -------------
            z2_ps = ps.tile([BC, H], F32, tag="z2", bufs=1)
            z2_mm = []
            for k in range(KT):
                z2_mm.append(
                    nc.tensor.matmul(z2_ps, h1[k], w2_sb[k],
                                     start=(k == 0),
                                     stop=(k == KT - 1 and not with_bias23)))
            # PE order: z1 fully before z2 (keeps tanh pipeline tight)
            add_dep_helper(z2_mm[0].ins, z1_mm[KT - 1].ins, sync=False,
                           reason="pe-order z2 after z1")
            if with_bias23:
                nc.tensor.matmul(z2_ps, onesr, b2r_sb, start=False, stop=True)
            h2 = ap.tile([BC, H], F32, tag="h2")
            s2 = ap.tile([BC, H], F32, tag="s2")
            for j in range(KT):
                nc.scalar.activation(h2[:, ts(j, 128)], z2_ps[:, ts(j, 128)],
                                     AF.Tanh)
                nc.gpsimd.tensor_mul(s2[:, ts(j, 128)], h2[:, ts(j, 128)],
                                     h2[:, ts(j, 128)])
                nc.gpsimd.tensor_scalar(s2[:, ts(j, 128)], s2[:, ts(j, 128)],
                                        -1.0, 1.0, ALU.mult, ALU.add)

            # ------------- trace weight matrix F -------------
            f_sb = []
            if USE_DIST_F:
                # each core computes 64 rows of E2T, all-gather the rest
                w1me_sb = cp.tile([D, D], F32, tag="w1me")
                nc.scalar.dma_start(out=w1me_sb, in_=w1me[:, :])
                e2t_ps = ps.tile([D, H], F32, tag="e2t", bufs=1)
                nc.tensor.matmul(e2t_ps, w1me_sb, w3T_sb,
                                 start=True, stop=True)
                e2t_sb = ap.tile([D, H], F32, tag="e2t_sb")
                nc.vector.tensor_copy(e2t_sb, e2t_ps)
                cc_in = dp.tile([D, H], F32, name="cc_in")
                cc_out = dp.tile([H, H], F32, name="cc_out")
                nc.sync.dma_start(out=cc_in, in_=e2t_sb)
                nc.gpsimd.collective_compute(
                    "AllGather", ALU.bypass,
                    replica_groups=[list(range(NCORES))],
                    ins=[cc_in.opt()], outs=[cc_out.opt()])
                for m in range(KT):
                    e2t_m = ap.tile([128, H], F32, tag=f"e2t_{m}",
                                    name=f"e2t_{m}")
                    nc.scalar.dma_start(out=e2t_m, in_=cc_out[ts(m, 128), :])
                    fm = ap.tile([128, H], F32, tag=f"f_{m}")
                    nc.vector.tensor_mul(fm, w2_sb[m], e2t_m)
                    f_sb.append(fm)
            else:
                for m in range(KT):
                    e2t_ps = ps.tile([128, H], F32, tag="e2t", bufs=2)
                    e2t_mm = nc.tensor.matmul(e2t_ps, w1x[:, ts(m, 128)],
                                              w3T_sb, start=True, stop=True)
                    add_dep_helper(e2t_mm.ins, z2_mm[0].ins, sync=False,
                                   reason="pe-order e2t after z2 starts")
                    fm = ap.tile([128, H], F32, tag=f"f_{m}")
                    nc.vector.tensor_mul(fm, w2_sb[m], e2t_ps)
                    f_sb.append(fm)

            # ------------- trJ = s1^T F s2 (batch-major) -------------
            t2_ps = ps.tile([BC, H], F32, tag="t2", bufs=1)
            for k in range(KT):
                nc.tensor.matmul(t2_ps, s1[k], f_sb[k],
                                 start=(k == 0), stop=(k == KT - 1))
            final_sb = ap.tile([BC, D + 1], F32, tag="final")
            ttr_scr = ap.tile([BC, H], F32, tag="ttr_scr")
            nc.vector.tensor_mul(ttr_scr, t2_ps, s2)
            nc.vector.tensor_reduce(out=final_sb[:, 0:1], in_=ttr_scr,
                                    op=ALU.add, axis=mybir.AxisListType.X,
                                    negate=True)

            # ------------- layer 3 (batch-major): dx -------------
            # per-j psum tiles (reuse the retired z1 slots) so the
            # transpose -> copy -> OUT chain pipelines without same-bank
            # serialization
            h2T_sb = []
            for j in range(KT):
                hT_ps = ps.tile([128, BC], F32, tag="z1", bufs=2)
                nc.tensor.transpose(hT_ps, h2[:, ts(j, 128)], ident)
                hT = ap.tile([128, BC], F32, tag=f"h2T_{j}", name=f"hT_{j}")
                nc.vector.tensor_copy(hT, hT_ps)
                h2T_sb.append(hT)
            o_ps = ps.tile([BC, D], F32, tag="o", bufs=1)
            for k in range(KT):
                nc.tensor.matmul(o_ps, h2T_sb[k], w3_sb[k],
                                 start=(k == 0),
                                 stop=(k == KT - 1 and not with_bias23))
            if with_bias23:
                nc.tensor.matmul(o_ps, onesr, b3r_sb, start=False, stop=True)
            nc.scalar.copy(final_sb[:, 1:D + 1], o_ps)
            nc.sync.dma_start(out=out[:, :], in_=final_sb)

    nc.finalize()
    return nc


def _get_nc(with_bias23: bool):
    key = bool(with_bias23)
    if key not in _NC:
        _NC[key] = _build(key)
    return _NC[key]


def make_in_maps(inputs):
    f32 = lambda a: np.ascontiguousarray(np.asarray(a), dtype=np.float32)
    state = f32(inputs["state"])
    t = float(np.asarray(inputs["t"]).reshape(-1)[0])
    W1 = f32(inputs["W1"])
    b1 = f32(inputs["b1"]).reshape(H)
    W2 = f32(inputs["W2"])
    b2 = f32(inputs["b2"]).reshape(H)
    W3 = f32(inputs["W3"])
    b3 = f32(inputs["b3"]).reshape(D)

    with_bias23 = bool(np.any(b2) or np.any(b3))

    b1_eff = b1 + t * W1[D]                       # fold t-row into bias
    cb = np.ascontiguousarray(b1_eff.reshape(KT, 128).T)

    base = {
        "W1x": np.ascontiguousarray(W1[:D]),
        "W2": W2,
        "W3": W3,
        "W3T": np.ascontiguousarray(W3.T),
        "cblk": cb,
    }
    if with_bias23:
        base["b2r"] = b2.reshape(1, H)
        base["b3r"] = b3.reshape(1, D)
    in_maps = []
    for c in range(NCORES):
        m = dict(base)
        m["stT"] = np.ascontiguousarray(state[c * BC:(c + 1) * BC, 1:].T)
        if USE_DIST_F:
            m["w1me"] = np.ascontiguousarray(W1[:D, c * D:(c + 1) * D])
        in_maps.append(m)
    return with_bias23, in_maps


def kernel(**inputs) -> np.ndarray:
    with_bias23, in_maps = make_in_maps(inputs)
    res = run_bass_kernel_spmd(_get_nc(with_bias23), in_maps,
                               list(range(NCORES))).results
    return np.concatenate([res[c]["out"] for c in range(NCORES)], axis=0)



# revision 3
# speedup vs baseline: 1.5569x; 1.5569x over previous
"""CNF forward (vector field + exact Jacobian trace) on 8 TRN2 cores.

Math: per sample x (row of state[:, 1:]):
    f(x)  = W3^T tanh(W2^T tanh(W1^T [x; t] + b1) + b2) + b3      (dx)
    trJ   = trace(df/dx)                                          (aug = -trJ)

Closed form of the trace (instead of D=64 JVPs per sample):
    h1 = tanh([x;t] @ W1 + b1),  h2 = tanh(h1 @ W2 + b2)
    s1 = 1 - h1^2,               s2 = 1 - h2^2
    trJ = s1^T F s2   with  F[i,j] = W2[i,j] * (W3 @ W1[:D])[j,i]

All matmul operands are bf16 (fp32 PSUM accumulation); fp32 matmul on
TRN2 runs as two LOW/HIGH passes with doubled LDWEIGHTS, so bf16 is
~4x tensor-engine time.  Rel-err budget is 2e-2; bf16 lands ~4e-3.

Layer 1 runs feature-major (z1T tiles into one PSUM bank) with bias
(b1 + t*W1[D]) folded in as a 65th contraction row, so one tanh ACT
covers the whole layer.  Layer 2 is batch-major from h1T tiles; the
trace tail is a single fused scalar_tensor_tensor:
    aug = sum((hh2 - 1) * t2, axis=1) = -s1^T F s2.
Layer 3 transposes h2 on the PE (identity matmul) and accumulates
o = h2 @ W3 batch-major.

Sharding: data-parallel, 128 samples per core, weights replicated.
Host-side work is layout-only (sharding, transposes, packing, dtype
casts); all FLOPs run on device.
"""

import numpy as np
import ml_dtypes

import concourse.bacc as bacc
import concourse.bass as bass
import concourse.tile as tile
from concourse import mybir
from concourse.bass_utils import run_bass_kernel_spmd
from concourse.masks import make_identity

B, D, H = 1024, 64, 512
NCORES = 8
BC = B // NCORES  # 128 samples per core
KT = H // 128     # 4 feature tiles of 128
F32 = mybir.dt.float32
BF16 = mybir.dt.bfloat16
AF = mybir.ActivationFunctionType
ALU = mybir.AluOpType
ts = bass.ts
BF_NP = ml_dtypes.bfloat16

_NC = {}

# blobA layout (bf16, [65, 1152]):
#   cols    0:512  W1A   (rows 0:64 = W1[:D], row 64 = b1 + t*W1[D])
#   cols  512:640  stT1  (rows 0:64 = x_core.T, row 64 = ones)
#   cols 640:1152  w3T   (rows 0:64 = W3.T, row 64 = zeros)
A_W1 = 0
A_ST = 512
A_W3T = 640
A_COLS = 1152


def _build(with_bias23: bool):
    nc = bacc.Bacc()

    blobA = nc.declare_dram_parameter("blobA", [D + 1, A_COLS], BF16,
                                      isOutput=False)
    # W2 row-blocks side by side: w2a = blocks 0,1; w2b = blocks 2,3
    w2a = nc.declare_dram_parameter("w2a", [128, 2 * H], BF16, isOutput=False)
    w2b = nc.declare_dram_parameter("w2b", [128, 2 * H], BF16, isOutput=False)
    # W3 row-blocks side by side: [128, 4*64]
    w3blk = nc.declare_dram_parameter("w3blk", [128, KT * D], BF16,
                                      isOutput=False)
    if with_bias23:
        b2r = nc.declare_dram_parameter("b2r", [1, H], BF16, isOutput=False)
        b3r = nc.declare_dram_parameter("b3r", [1, D], BF16, isOutput=False)
    out = nc.declare_dram_parameter("out", [BC, D + 1], F32, isOutput=True)

    with tile.TileContext(nc) as tc:
        with (
            tc.tile_pool(name="const", bufs=1) as cp,
            tc.tile_pool(name="act", bufs=1) as ap,
            tc.tile_pool(name="ps", bufs=1, space="PSUM") as ps,
        ):
            # ------------- loads -------------
            a_sb = cp.tile([D + 1, A_COLS], BF16, tag="a_sb")
            w2a_sb = cp.tile([128, 2 * H], BF16, tag="w2a")
            w2b_sb = cp.tile([128, 2 * H], BF16, tag="w2b")
            w3_sb = cp.tile([128, KT * D], BF16, tag="w3")
            nc.scalar.dma_start(out=a_sb, in_=blobA[:, :])
            nc.sync.dma_start(out=w2a_sb, in_=w2a[:, :])
            nc.sync.dma_start(out=w2b_sb, in_=w2b[:, :])
            nc.sync.dma_start(out=w3_sb, in_=w3blk[:, :])
            if with_bias23:
                b2r_sb = cp.tile([1, H], BF16, tag="b2r")
                nc.sync.dma_start(out=b2r_sb, in_=b2r[:, :])
                b3r_sb = cp.tile([1, D], BF16, tag="b3r")
                nc.sync.dma_start(out=b3r_sb, in_=b3r[:, :])
                onesr = cp.tile([1, BC], BF16, tag="onesr")
                nc.gpsimd.memset(onesr, 1.0)
            ident = cp.tile([128, 128], BF16, tag="ident")
            make_identity(nc, ident)

            def w2s(k):
                return (w2a_sb if k < 2 else w2b_sb)[:, ts(k % 2, H)]

            # ------------- layer 1 (feature-major, one PSUM bank) ----
            z1_ps = ps.tile([128, H], F32, tag="z1")
            for j in range(KT):
                nc.tensor.matmul(z1_ps[:, ts(j, 128)],
                                 a_sb[:, A_W1 + j * 128:A_W1 + (j + 1) * 128],
                                 a_sb[:, A_ST:A_ST + BC],
                                 start=True, stop=True)
            h1T = ap.tile([128, H], BF16, tag="h1T")
            nc.scalar.activation(h1T, z1_ps, AF.Tanh)
            hh1 = ap.tile([128, H], BF16, tag="hh1")
            nc.gpsimd.tensor_mul(hh1, h1T, h1T)
            s1T = ap.tile([128, H], BF16, tag="s1T")
            nc.gpsimd.tensor_scalar(s1T, hh1, -1.0, 1.0, ALU.mult, ALU.add)

            # ------------- trace weight matrix F ---------------------
            f_sb = []
            for m in range(KT):
                e2t_ps = ps.tile([128, H], F32, tag="e2t", bufs=2)
                nc.tensor.matmul(e2t_ps,
                                 a_sb[0:D, A_W1 + m * 128:A_W1 + (m + 1) * 128],
                                 a_sb[0:D, A_W3T:A_W3T + H],
                                 start=True, stop=True)
                fm = ap.tile([128, H], BF16, tag=f"f_{m}")
                nc.vector.tensor_mul(fm, w2s(m), e2t_ps)
                f_sb.append(fm)

            # ------------- layer 2 (batch-major) ---------------------
            z2_ps = ps.tile([BC, H], F32, tag="z2")
            for k in range(KT):
                nc.tensor.matmul(z2_ps, h1T[:, ts(k, 128)], w2s(k),
                                 start=(k == 0),
                                 stop=(k == KT - 1 and not with_bias23))
            if with_bias23:
                nc.tensor.matmul(z2_ps, onesr, b2r_sb, start=False, stop=True)
            h2 = ap.tile([BC, H], BF16, tag="h2")
            nc.scalar.activation(h2, z2_ps, AF.Tanh)
            hh2 = ap.tile([BC, H], BF16, tag="hh2")
            nc.vector.tensor_mul(hh2, h2, h2)

            # ------------- trJ = s1^T F s2 (fused tail) --------------
            t2_ps = ps.tile([BC, H], F32, tag="t2")
            for k in range(KT):
                nc.tensor.matmul(t2_ps, s1T[:, ts(k, 128)], f_sb[k],
                                 start=(k == 0), stop=(k == KT - 1))
            final_sb = ap.tile([BC, D + 1], F32, tag="final")
            stt_scr = ap.tile([BC, H], F32, tag="stt_scr")
            # aug = sum((hh2 - 1) * t2) = -s1^T F s2
            nc.vector.scalar_tensor_tensor(
                out=stt_scr, in0=hh2, scalar=1.0, in1=t2_ps,
                op0=ALU.subtract, op1=ALU.mult,
                accum_out=final_sb[:, 0:1])

            # ------------- layer 3 (batch-major via PE transpose) ----
            h2T_sb = []
            for j in range(KT):
                hT_ps = ps.tile([128, BC], BF16, tag="tr", bufs=2)
                nc.tensor.transpose(hT_ps, h2[:, ts(j, 128)], ident)
                hT = ap.tile([128, BC], BF16, tag=f"h2T_{j}")
                if j % 2 == 0:
                    nc.vector.tensor_copy(hT, hT_ps)
                else:
                    nc.scalar.copy(hT, hT_ps)
                h2T_sb.append(hT)
            o_ps = ps.tile([BC, D], F32, tag="o")
            for k in range(KT):
                nc.tensor.matmul(o_ps, h2T_sb[k], w3_sb[:, ts(k, D)],
                                 start=(k == 0),
                                 stop=(k == KT - 1 and not with_bias23))
            if with_bias23:
                nc.tensor.matmul(o_ps, onesr, b3r_sb, start=False, stop=True)
            nc.scalar.copy(final_sb[:, 1:D + 1], o_ps)
            nc.sync.dma_start(out=out[:, :], in_=final_sb)

    nc.finalize()
    return nc


def _get_nc(with_bias23: bool):
    key = bool(with_bias23)
    if key not in _NC:
        _NC[key] = _build(key)
    return _NC[key]


def make_in_maps(inputs):
    f32 = lambda a: np.ascontiguousarray(np.asarray(a), dtype=np.float32)
    bf = lambda a: np.ascontiguousarray(np.asarray(a, dtype=np.float32)
                                        .astype(BF_NP))
    state = f32(inputs["state"])
    t = float(np.asarray(inputs["t"]).reshape(-1)[0])
    W1 = f32(inputs["W1"])
    b1 = f32(inputs["b1"]).reshape(H)
    W2 = f32(inputs["W2"])
    b2 = f32(inputs["b2"]).reshape(H)
    W3 = f32(inputs["W3"])
    b3 = f32(inputs["b3"]).reshape(D)

    with_bias23 = bool(np.any(b2) or np.any(b3))

    W1A = np.concatenate([W1[:D], (b1 + t * W1[D])[None, :]], axis=0)  # [65,H]
    w3T_pad = np.concatenate([W3.T, np.zeros((1, H), np.float32)], axis=0)
    w2a = np.concatenate([W2[0:128], W2[128:256]], axis=1)      # [128, 1024]
    w2b = np.concatenate([W2[256:384], W2[384:512]], axis=1)    # [128, 1024]
    w3b = np.concatenate([W3[k * 128:(k + 1) * 128] for k in range(KT)],
                         axis=1)                                # [128, 256]

    base = {"w2a": bf(w2a), "w2b": bf(w2b), "w3blk": bf(w3b)}
    if with_bias23:
        base["b2r"] = bf(b2.reshape(1, H))
        base["b3r"] = bf(b3.reshape(1, D))

    x = state[:, 1:]
    in_maps = []
    for c in range(NCORES):
        stT1 = np.concatenate([x[c * BC:(c + 1) * BC].T,
                               np.ones((1, BC), np.float32)], axis=0)
        blobA = np.concatenate([W1A, stT1, w3T_pad], axis=1)    # [65, 1152]
        m = dict(base)
        m["blobA"] = bf(blobA)
        in_maps.append(m)
    return with_bias23, in_maps


def kernel(**inputs) -> np.ndarray:
    with_bias23, in_maps = make_in_maps(inputs)
    res = run_bass_kernel_spmd(_get_nc(with_bias23), in_maps,
                               list(range(NCORES))).results
    return np.concatenate([res[c]["out"] for c in range(NCORES)], axis=0)
